# revision 66
# baseline (speedup 1.0000x reference)
"""Trainium2 Bass kernel for nn_CCMetrics (connected-component soft-Dice).

Math
----
Reference per sample: probs = softmax(y_pred, ch axis 1) with C=2 channels,
one-hot labels y in {0,1}.  Per-voxel channel sums collapse:
  psum_v = tsum_v = 1          (softmax / one-hot sum to 1 over channels)
  inter_v = probs[true_ch] = sigmoid((2y-1) * (z1 - z0))
So per segment id k (voronoi component, 0..64):
  inter_k = sum of sigmoid values over voxels with id k
  cnt_k   = voxel count with id k
  dice_k  = (2*inter_k + eps) / (2*cnt_k + eps)
  score   = mean over present k in 1..64;  output = mean over batch.

Device algorithm (per core, data-parallel over 4M voxels / 8 cores)
------------------------------------------------------------------
Packed stream per voxel (id g, value v = sigmoid(+-(z1-z0))):
  ztx = 2g + 1                  (odd integers, exact in fp16)
  X   = ztx + v   in fp16       (v lands in the gap [2g+1, 2g+2];
                                 counts stay exact, v quantized ~1/16)
Two per-bin families, k = 1..64:
  R_k = sum relu(X - (2k+1))    value family; M_k = R_k - R_{k+1}
                                 = inter~_k + 2*T_{k+1}
  T_k = #{X >= 2k} = #{g >= k}  count family (exact)
Work is split over THREE engines (the baseline used only ACT+DVE):
  * ACT: self-contained accumulate passes: Relu on X for R_k, and
    steep sigmoid(20*(ztx - 2k)) on ztx for T_k (exact saturation).
  * DVE: preprocessing + cheap 4x fp16 tensor_scalar tiles:
    is_ge masks (counts) and sub+max relu tiles (values).
  * PE (tensor engine, idle in baseline): reduces each DVE tile with
    4 matmuls (moving 1024 cols) against a one-hot [128,64] stationary,
    accumulating every bin into shared PSUM tiles [64, 1024].
Host recovers inter/cnt from R/T in float64 and finishes the dice mean.
"""

import os
import sys

import numpy as np

for _p in ("/opt/trn_rl_repo",):
    if os.path.isdir(_p) and _p not in sys.path:
        sys.path.insert(0, _p)

from concourse import bacc, bass, mybir, tile  # noqa: E402
from concourse import bass_utils  # noqa: E402

NUM_COMP = 64
EPS = 1e-5
B, C, H, W, D = 2, 2, 128, 128, 128
N = H * W * D
NCORES = 8
CORES_PER_SAMPLE = NCORES // B
CHUNK = N // CORES_PER_SAMPLE
P = 128
F = CHUNK // P          # 4096
K = NUM_COMP            # 64 foreground bins
NCH = 8                 # moving chunks per PE reduce
FCH = F // NCH          # 512 (max moving free dim per matmul)

# ---- engine assignment knobs (tunable via env) ----
_ACT_CNT = int(os.environ.get("CC_ACT_CNT", "6"))   # count units on ACT
_ACT_VAL = int(os.environ.get("CC_ACT_VAL", "28"))  # value units on ACT
_ACT_HEAD = int(os.environ.get("CC_ACT_HEAD", "6"))  # ACT cnt units before v
_DVE_HEAD = int(os.environ.get("CC_DVE_HEAD", "8"))  # PE cnt gens before d
_NFOLD = int(os.environ.get("CC_FOLD", "0"))  # PE cnt units w/ DVE pre-fold
_MBUFS = int(os.environ.get("CC_MBUFS", "5"))  # mask buffers per family
_NC8 = int(os.environ.get("CC_C8", "18"))  # PE count units in fp8+DoubleRow
_NV8 = int(os.environ.get("CC_V8", "0"))   # PE value units in fp8+DoubleRow
_NGP = int(os.environ.get("CC_GP", "0"))   # count units generated on GPSIMD
# (measured: gpsimd TENSOR_SCALAR ucode runs ~64us per [128,4096] tile --
# not viable as a mask generator; keep 0)
_DMA_PRE = os.environ.get("CC_DMA_PRE", "1") == "1"  # t via SWDGE cast-DMA

ACT_CNT_KS = list(range(1, _ACT_CNT + 1))
PE_CNT_KS = list(range(_ACT_CNT + 1, K + 1))
ACT_VAL_KS = list(range(1, _ACT_VAL + 1))
PE_VAL_KS = list(range(_ACT_VAL + 1, K + 1))

TRACE = False

_prog_cache = {}


def _build_program():
    nc = bacc.Bacc(
        "TRN2",
        target_bir_lowering=False,
        debug=False,
        enable_asserts=False,
        num_devices=NCORES,
    )
    f32 = mybir.dt.float32
    f16 = mybir.dt.float16

    # za = (2y-1)*z1, zb = -(2y-1)*z0  (host-side sign flips, bit-exact),
    # so t = za + zb = (2y-1)*(z1-z0) comes out of one accumulate-DMA
    z0_d = nc.dram_tensor("za", [P, F], f32, kind="ExternalInput").ap()
    z1_d = nc.dram_tensor("zb", [P, F], f32, kind="ExternalInput").ap()
    yf_d = nc.dram_tensor("yf", [P, F], f16, kind="ExternalInput").ap()
    ztx_d = nc.dram_tensor("ztx", [P, F], f16, kind="ExternalInput").ap()
    oneh_d = nc.dram_tensor("oneh", [P, K * K], f16, kind="ExternalInput").ap()
    f8 = mybir.dt.float8e4
    # doubled one-hot for DoubleRow count reduces: unit row j has ones at
    # cols 2K*j + j and 2K*j + K + j
    oneh8_d = nc.dram_tensor("oneh8", [P, 2 * K * K], f8, kind="ExternalInput").ap()
    # bias constants: cols 0..63 relu bias -(2k+1); cols 64..127 sigmoid
    # bias -40k; col 128: 0.0
    bias_d = nc.dram_tensor("bias", [P, 2 * K + 1], f32, kind="ExternalInput").ap()
    racc_d = nc.dram_tensor("racc", [P, K], f32, kind="ExternalOutput").ap()
    facc_d = nc.dram_tensor("facc", [P, K], f32, kind="ExternalOutput").ap()
    pval_d = nc.dram_tensor("pval", [K, FCH], f32, kind="ExternalOutput").ap()
    # fp8 DoubleRow count units write [64, FCH//2]; fp16 ones [64, FCH].
    # Use separate PSUM regions per width.
    pcnt_d = nc.dram_tensor("pcnt", [K, FCH], f32, kind="ExternalOutput").ap()
    pcnt8_d = nc.dram_tensor("pcnt8", [K, FCH // 2], f32, kind="ExternalOutput").ap()
    pval8_d = nc.dram_tensor("pval8", [K, FCH // 2], f32, kind="ExternalOutput").ap()

    Alu = mybir.AluOpType
    Act = mybir.ActivationFunctionType

    with tile.TileContext(nc) as tc:
        with tc.tile_pool(name="main", bufs=1) as pool, \
             tc.tile_pool(name="mask", bufs=1) as mpool, \
             tc.tile_pool(name="psum", bufs=1, space="PSUM") as ppool:
            ztx = pool.tile([P, F], f16)
            oneh = pool.tile([P, K * K], f16)
            oneh8 = pool.tile([P, 2 * K * K], f8)
            bias = pool.tile([P, 2 * K + 1], f32)
            # small tensors first so early ACT/DVE work is not gated on the
            # 4MB z DMAs
            nc.sync.dma_start(out=ztx[:], in_=ztx_d[:])
            nc.sync.dma_start(out=bias[:], in_=bias_d[:])
            nc.sync.dma_start(out=oneh[:], in_=oneh_d[:])
            if _NC8 > 0:
                nc.sync.dma_start(out=oneh8[:], in_=oneh8_d[:])
            t16 = pool.tile([P, F], f16)
            if _DMA_PRE:
                # SWDGE cast-DMAs: za/zb arrive as fp16, so the t add runs
                # at DVE 2x instead of fp32 1x
                za16 = pool.tile([P, F], f16)
                zb16 = pool.tile([P, F], f16)
                nc.gpsimd.dma_start(out=za16[:], in_=z0_d[:])
                nc.gpsimd.dma_start(out=zb16[:], in_=z1_d[:])
            else:
                yf = pool.tile([P, F], f16)
                z0 = pool.tile([P, F], f32)
                z1 = pool.tile([P, F], f32)
                nc.sync.dma_start(out=yf[:], in_=yf_d[:])
                nc.sync.dma_start(out=z0[:], in_=z0_d[:])
                nc.sync.dma_start(out=z1[:], in_=z1_d[:])

            pv = ppool.tile([K, FCH], f32)
            pc = ppool.tile([K, FCH], f32)
            pc8 = ppool.tile([K, FCH // 2], f32)
            pv8 = ppool.tile([K, FCH // 2], f32)
            racc = pool.tile([P, K], f32)
            facc = pool.tile([P, K], f32)
            trash_a = pool.tile([P, F], f16)

            # --- PE accumulation bookkeeping: start=True only on the very
            # first matmul touching a region (resets PSUM), stop=True on the
            # very last (sim requirement). ---
            nv8 = min(_NV8, len(PE_VAL_KS))
            ngp = min(_NGP, len(PE_CNT_KS))
            nc8 = min(_NC8, len(PE_CNT_KS) - ngp)
            # gpsimd units take the front ks, fp8 units the back ks
            gp_ks = set(PE_CNT_KS[:ngp]) if ngp else set()
            fp8_ks = set(PE_CNT_KS[-nc8:]) if nc8 else set()
            fp8_vks = set(PE_VAL_KS[-nv8:]) if nv8 else set()
            n_pe_mm = {id(pv): 0, id(pc): 0, id(pc8): 0, id(pv8): 0}
            tot_pe_mm = {
                id(pv): (len(PE_VAL_KS) - nv8) * NCH,
                id(pc): (len(PE_CNT_KS) - nc8 - ngp) * NCH,
                id(pc8): (nc8 + ngp) * NCH,
                id(pv8): nv8 * NCH,
            }

            def pe_reduce(m, region, row, nch):
                fch = F // nch
                lhs = oneh[:, K * row:K * row + K]
                for c in range(nch):
                    i = n_pe_mm[id(region)]
                    nc.tensor.matmul(
                        region[:],
                        lhs,
                        m[:, c * fch:(c + 1) * fch],
                        start=(i == 0),
                        stop=(i == tot_pe_mm[id(region)] - 1),
                        skip_group_check=True,
                    )
                    n_pe_mm[id(region)] += 1

            def pe_reduce_dr(m, region, row):
                # fp8 DoubleRow: each matmul consumes a [128, 2, FCH//2]
                # moving view (2 fp8 elems/cell/cycle), halving PE time
                lhs = oneh8[:, 2 * K * row:2 * K * (row + 1)].rearrange(
                    "p (o m) -> p o m", o=2)
                for c in range(NCH):
                    i = n_pe_mm[id(region)]
                    rhs = m[:, c * FCH:(c + 1) * FCH].rearrange(
                        "p (o n) -> p o n", o=2)
                    nc.tensor.matmul(
                        region[:],
                        lhs,
                        rhs,
                        start=(i == 0),
                        stop=(i == tot_pe_mm[id(region)] - 1),
                        perf_mode=mybir.MatmulPerfMode.DoubleRow,
                        skip_group_check=True,
                    )
                    n_pe_mm[id(region)] += 1

            def act_cnt(k):
                nc.scalar.activation(
                    out=trash_a[:], in_=ztx[:], func=Act.Sigmoid,
                    bias=bias[:, K + k - 1:K + k], scale=20.0,
                    accum_out=facc[:, k - 1:k],
                )

            def act_val(k):
                nc.scalar.activation(
                    out=trash_a[:], in_=x[:], func=Act.Relu,
                    bias=bias[:, k - 1:k], scale=1.0,
                    accum_out=racc[:, k - 1:k],
                )

            def gen_cnt(k):
                use8 = k in fp8_ks
                m = mpool.tile([P, F], f8 if use8 else f16,
                               tag="c8mask" if use8 else "cmask",
                               bufs=3 if use8 else (_MBUFS - 1),
                               name=f"cm{k}")
                nc.vector.tensor_scalar(
                    out=m[:], in0=ztx[:], scalar1=float(2 * k), scalar2=None,
                    op0=Alu.is_ge,
                )
                if use8:
                    pe_reduce_dr(m, pc8, k - 1)
                else:
                    pe_reduce(m, pc, k - 1, NCH)

            def gen_cnt_gp(k):
                # is_ge on GPSIMD (its own engine queue) -> fp8 mask for a
                # DoubleRow PE reduce; a 4th worker alongside ACT/DVE/PE
                m = mpool.tile([P, F], f8, tag="gmask", bufs=4,
                               name=f"gm{k}")
                nc.gpsimd.tensor_scalar(
                    out=m[:], in0=ztx[:], scalar1=float(2 * k), scalar2=None,
                    op0=Alu.is_ge,
                )
                pe_reduce_dr(m, pc8, k - 1)

            def gen_val(k):
                if k in fp8_vks:
                    # masked-value tile (ztx>=2k)*v in [0,1]: fp8-safe, so
                    # the PE reduce can run DoubleRow; gives V_k directly
                    m = mpool.tile([P, F], f8, tag="v8mask", bufs=3,
                                   name=f"vm8{k}")
                    nc.vector.scalar_tensor_tensor(
                        out=m[:], in0=ztx[:], scalar=float(2 * k), in1=v16[:],
                        op0=Alu.is_ge, op1=Alu.mult,
                    )
                    pe_reduce_dr(m, pv8, k - 1)
                    return
                m = mpool.tile([P, F], f16, tag="vmask", bufs=_MBUFS,
                               name=f"vm{k}")
                nc.vector.tensor_scalar(
                    out=m[:], in0=x[:], scalar1=float(2 * k + 1), scalar2=0.0,
                    op0=Alu.subtract, op1=Alu.max,
                )
                pe_reduce(m, pv, k - 1, NCH)

            # ---- emission order = per-engine FIFO order ----
            # GPSIMD count-mask gens all emitted up front (own engine queue,
            # starts as soon as ztx lands); their PE reduces are interleaved
            # into the PE stream later
            gp_pending = []
            for k in sorted(gp_ks):
                m = mpool.tile([P, F], f8, tag="gmask", bufs=4,
                               name=f"gm{k}")
                nc.gpsimd.tensor_scalar(
                    out=m[:], in0=ztx[:], scalar1=float(2 * k), scalar2=None,
                    op0=Alu.is_ge,
                )
                gp_pending.append((k, m))
            # ACT head: sigmoid count units (need only ztx)
            for k in ACT_CNT_KS[:_ACT_HEAD]:
                act_cnt(k)
            # DVE head: count-mask gens + PE reduces (need only ztx) to keep
            # DVE/PE busy while the z DMAs stream in
            rem_cnt = [k for k in PE_CNT_KS if k not in gp_ks]
            for k in rem_cnt[:_DVE_HEAD]:
                gen_cnt(k)
            if _DMA_PRE:
                nc.vector.tensor_add(t16[:], za16[:], zb16[:])
            else:
                # fallback preprocessing on DVE: t = za + zb
                nc.vector.tensor_add(t16[:], z1[:], z0[:])
            # v = sigmoid(t) on ACT (after its head count units)
            v16 = pool.tile([P, F], f16)
            nc.scalar.activation(
                out=v16[:], in_=t16[:], func=Act.Sigmoid,
                bias=bias[:, 2 * K:2 * K + 1], scale=1.0,
            )
            # a couple more DVE count gens while ACT computes v
            for k in rem_cnt[_DVE_HEAD:_DVE_HEAD + 2]:
                gen_cnt(k)
            # X = ztx + v
            x = pool.tile([P, F], f16)
            nc.vector.tensor_add(x[:], ztx[:], v16[:])
            # remaining ACT units: counts first (no X dep), then values
            for k in ACT_CNT_KS[_ACT_HEAD:]:
                act_cnt(k)
            for k in ACT_VAL_KS:
                act_val(k)
            # remaining DVE->PE units: interleave values with leftover counts
            rest_cnt = rem_cnt[_DVE_HEAD + 2:]
            rest_val = list(PE_VAL_KS)
            seq = []
            nc_, nv_ = len(rest_cnt), len(rest_val)
            ci = vi = 0
            for i in range(nc_ + nv_):
                # spread counts evenly among values
                if ci < nc_ and (vi >= nv_ or ci * nv_ <= vi * nc_):
                    seq.append(("c", rest_cnt[ci])); ci += 1
                else:
                    seq.append(("v", rest_val[vi])); vi += 1
            # spread the gpsimd-mask PE reduces uniformly through the stream
            ngp_ = len(gp_pending)
            gp_queue = list(gp_pending)
            for si, (kind, k) in enumerate(seq):
                while gp_queue and len(gp_queue) > ngp_ * (len(seq) - si) / max(len(seq), 1):
                    gk, gm = gp_queue.pop(0)
                    pe_reduce_dr(gm, pc8, gk - 1)
                if kind == "c":
                    gen_cnt(k)
                else:
                    gen_val(k)
            for gk, gm in gp_queue:
                pe_reduce_dr(gm, pc8, gk - 1)

            # ---- drain: PSUM -> SBUF -> DRAM, ACT accumulators -> DRAM ----
            pv_sb = pool.tile([K, FCH], f32)
            pc_sb = pool.tile([K, FCH], f32)
            pc8_sb = pool.tile([K, FCH // 2], f32)
            nc.vector.tensor_copy(pv_sb[:], pv[:])
            nc.vector.tensor_copy(pc_sb[:], pc[:])
            nc.sync.dma_start(out=pval_d[:], in_=pv_sb[:])
            nc.sync.dma_start(out=pcnt_d[:], in_=pc_sb[:])
            if nc8:
                nc.vector.tensor_copy(pc8_sb[:], pc8[:])
                nc.sync.dma_start(out=pcnt8_d[:], in_=pc8_sb[:])
            if nv8:
                pv8_sb = pool.tile([K, FCH // 2], f32)
                nc.vector.tensor_copy(pv8_sb[:], pv8[:])
                nc.sync.dma_start(out=pval8_d[:], in_=pv8_sb[:])
            nc.sync.dma_start(out=racc_d[:], in_=racc[:])
            nc.sync.dma_start(out=facc_d[:], in_=facc[:])

    nc.compile()
    return nc


def _get_program():
    key = ("prog", _ACT_CNT, _ACT_VAL, _ACT_HEAD, _DVE_HEAD, _NFOLD, _MBUFS,
           _NC8, _NV8, _DMA_PRE)
    if key not in _prog_cache:
        _prog_cache[key] = _build_program()
    return _prog_cache[key]


def _onehot_const():
    oneh = np.zeros((P, K * K), dtype=np.float16)
    for j in range(K):
        oneh[:, K * j + j] = 1.0
    return oneh


def _onehot8_const():
    import ml_dtypes
    oneh8 = np.zeros((P, 2 * K * K), dtype=ml_dtypes.float8_e4m3)
    for j in range(K):
        oneh8[:, 2 * K * j + j] = 1.0
        oneh8[:, 2 * K * j + K + j] = 1.0
    return oneh8


def _bias_const():
    ks = np.arange(1, K + 1, dtype=np.float32)
    row = np.concatenate([-(2 * ks + 1), -40.0 * ks, np.zeros(1, np.float32)])
    return np.broadcast_to(row, (P, 2 * K + 1)).copy()


def kernel(y_pred: np.ndarray, y: np.ndarray, voronoi: np.ndarray) -> np.ndarray:
    y_pred = np.asarray(y_pred, dtype=np.float32)
    y = np.asarray(y)
    voronoi = np.asarray(voronoi)

    nc = _get_program()
    oneh = _onehot_const()
    oneh8 = _onehot8_const()
    biasc = _bias_const()

    in_maps = []
    for c in range(NCORES):
        b = c // CORES_PER_SAMPLE
        q = c % CORES_PER_SAMPLE
        sl = slice(q * CHUNK, (q + 1) * CHUNK)
        zp = y_pred[b].reshape(C, N)
        yv = y[b, 0].reshape(N)[sl].astype(bool)
        gv = voronoi[b].reshape(N)[sl].astype(np.int32)
        z0c = zp[0, sl]
        z1c = zp[1, sl]
        # bit-exact sign flips folding the (2y-1) factor into the z inputs
        za = np.where(yv, z1c, -z1c).reshape(P, F)
        zb = np.where(yv, -z0c, z0c).reshape(P, F)
        in_maps.append({
            "za": np.ascontiguousarray(za),
            "zb": np.ascontiguousarray(zb),
            "yf": (2 * yv.astype(np.int32) - 1).astype(np.float16).reshape(P, F),
            "ztx": (2 * gv + 1).astype(np.float16).reshape(P, F),
            "oneh": oneh,
            "oneh8": oneh8,
            "bias": biasc,
        })

    res = bass_utils.run_bass_kernel_spmd(
        nc, in_maps, core_ids=list(range(NCORES)), trace=TRACE,
    )
    kernel.last_results = res

    # ---- host-side gather/unshard: combine per-core partials (float64) ----
    nc8 = min(_NC8, len(PE_CNT_KS))
    nv8 = min(_NV8, len(PE_VAL_KS))
    fp8_ks = set(PE_CNT_KS[-nc8:]) if nc8 else set()
    fp8_vks = set(PE_VAL_KS[-nv8:]) if nv8 else set()
    R = np.zeros((B, K + 2), dtype=np.float64)   # relu family
    V = np.zeros((B, K + 2), dtype=np.float64)   # masked-value family
    T = np.zeros((B, K + 2), dtype=np.float64)
    for c in range(NCORES):
        b = c // CORES_PER_SAMPLE
        out = res.results[c]
        racc = np.asarray(out["racc"], dtype=np.float64)
        facc = np.asarray(out["facc"], dtype=np.float64)
        pval = np.asarray(out["pval"], dtype=np.float64)
        pcnt = np.asarray(out["pcnt"], dtype=np.float64)
        pcnt8 = np.asarray(out["pcnt8"], dtype=np.float64)
        pval8 = np.asarray(out["pval8"], dtype=np.float64)
        for k in ACT_VAL_KS:
            R[b, k] += racc[:, k - 1].sum()
        for k in PE_VAL_KS:
            if k in fp8_vks:
                V[b, k] += pval8[k - 1, :].sum()
            else:
                R[b, k] += pval[k - 1, :].sum()
        for k in ACT_CNT_KS:
            T[b, k] += facc[:, k - 1].sum()
        for k in PE_CNT_KS:
            src = pcnt8 if k in fp8_ks else pcnt
            T[b, k] += src[k - 1, :].sum()

    scores = []
    ks = np.arange(1, K + 1)
    for b in range(B):
        # suffix count sums: S[k] = sum_{j>=k} T_j  (S[65] = S[66] = 0)
        S = np.zeros(K + 3, dtype=np.float64)
        for k in range(K, 0, -1):
            S[k] = S[k + 1] + T[b, k]
        # unify value families: V_k = R_k - 2*S_{k+1} for relu-sourced bins
        Vk = np.zeros(K + 2, dtype=np.float64)
        for k in range(1, K + 1):
            if k in fp8_vks:
                Vk[k] = V[b, k]
            else:
                Vk[k] = R[b, k] - 2.0 * S[k + 1]
        inter = Vk[ks] - Vk[ks + 1]          # V[65] = 0
        cnt = np.round(T[b, ks] - T[b, ks + 1])
        dice = (2.0 * inter + EPS) / (2.0 * cnt + EPS)
        present = cnt > 0
        n_present = max(present.sum(), 1)
        scores.append(np.where(present, dice, 0.0).sum() / n_present)

    return np.float32(np.mean(scores))
